# revision 34
# baseline (speedup 1.0000x reference)
"""Multi-headed attention (pre-LN, quirk-wired) Trainium2 Bass kernel.

Sharding: 8 cores = 2 batches x 4 head-groups (4 heads each).
I/O-lean contract (host<->device transfer dominates the wall clock, so
ship the information-theoretic minimum): each core ships its strided
token-quarter of k/q as uint8 (LayerNorm is invariant to the per-token
quantization affine, so no scales ship and nothing dequantizes), v as
bf16 (its error hits the output directly via the residual), a
bit-packed mask^T quarter, half of its head-slice weights, and folded
LN/bias constants — small tensors merged into one aux byte-tensor via
bitcast views.  Full per-batch tensors are rebuilt on device with
AllGathers (batch group for x/mask, cross-batch pair for weights).
Canonical 128-token tiles map to contiguous blocks of the gathered
buffer (block b = (tt%4)*4 + tt//4), so the attention math is
unchanged from the dense version.  Output returns as bf16.

Per core: full-batch LN-folded projections (its head slice), scores^T
layout attention with ones-column softmax denominators, Wo partial,
ReduceScatter over the 4-core batch group, residual added post-RS.

reference semantics:
  kn,qn,vn = LN(k),LN(q),LN(v)   (ddof=1 std, eps added to std, affine a2,b2)
  query = kn@Wq+bq ; key = qn@Wk+bk ; value = vn@Wv+bv   (note stream quirk)
  out = softmax(mask(QK^T/8)) @ V  -> @Wo + bo + vn
"""
import math
import numpy as np
import ml_dtypes

import jax as _jax
try:
    _jax.config.update("jax_compilation_cache_dir", "/tmp/jax_cc_cache")
    _jax.config.update("jax_persistent_cache_min_compile_time_secs", 0)
    _jax.config.update("jax_persistent_cache_min_entry_size_bytes", 0)
except Exception:
    pass

import concourse.bass as bass
import concourse.tile as tile
from concourse import bacc, mybir
from concourse.bass_utils import run_bass_kernel_spmd
from concourse.masks import make_identity

BF = ml_dtypes.bfloat16
B, S, D, H = 2, 2048, 1024, 16
DK = D // H            # 64
NCORES = 8
HG = 4                 # head-groups per batch
HPG = H // HG          # 4 heads per core
DHG = HPG * DK         # 256 head-dim slice per core
EPS = 1e-6
P = 128
NTT = S // P           # 16 token tiles
NQS = 4                # query slices of 512
QS = S // NQS          # 512
VAR_C = D / (D - 1.0)  # ddof=1 correction
GROUPS = [[0, 1, 2, 3], [4, 5, 6, 7]]
PAIRS = [[0, 4], [1, 5], [2, 6], [3, 7]]   # same head-slice, both batches
WSZ = D * DHG          # elements of one projection weight slice

_CACHE = {}


def _build(has_a2, has_bias_out):
    nc = bacc.Bacc("TRN2", target_bir_lowering=False, debug=False,
                   num_devices=NCORES)
    f32, bf16 = mybir.dt.float32, mybir.dt.bfloat16
    u8 = mybir.dt.uint8
    dram_in = {}
    for nm, shape, dt in [
        ("xsl0", [QS, D], u8),             # strided token-quarter, k stream
        ("xsl1", [QS, D], u8),             # .. q stream  (uint8: LN absorbs
                                           #  the per-token quant affine)
        ("xsl2", [QS, D], bf16),           # .. v stream (bf16: residual +
                                           #  PV values dominate the error)
        # aux = bf16 [whalf(2,WSZ) | cpk(2,3*DHG+D)] bytes ++ u8 maskTp bytes
        ("aux", [2 * (2 * WSZ + 2 * (3 * DHG + D)) + QS * (S // 8)], u8),
    ]:
        dram_in[nm] = nc.dram_tensor(nm, shape, dt, kind="ExternalInput").ap()
    out_shard = nc.dram_tensor("out_shard", [NQS, P, D], bf16,
                               kind="ExternalOutput").ap()
    NBF = 2 * WSZ + 2 * (3 * DHG + D)      # bf16 elements in aux head
    aux_bf = dram_in["aux"][0:2 * NBF].bitcast(bf16)
    whalf_v = aux_bf[0:2 * WSZ].rearrange("(a b) -> a b", a=2)
    cpk_v = aux_bf[2 * WSZ:NBF].rearrange("(a b) -> a b", a=2)
    maskTp_v = dram_in["aux"][2 * NBF:].rearrange("(a b) -> a b", a=QS)

    with tile.TileContext(nc, trace_sim=False) as tc:
        with tc.tile_pool(name="const", bufs=1) as constp, \
             tc.tile_pool(name="persist", bufs=1) as persist, \
             tc.tile_pool(name="dram", bufs=1, space="DRAM") as dramp:

            # ---- gather weights, full-batch activations, mask on device ---
            wb = dramp.tile([2, WSZ], bf16, tag="wb")
            xin_b = dramp.tile([2, QS, D], u8, tag="xin_b")
            xvb = dramp.tile([QS, D], bf16, tag="xvb")
            mb = dramp.tile([QS, S // 8], u8, tag="mb")
            wg = dramp.tile([4, WSZ], bf16, tag="wg")
            xg = dramp.tile([HG, 2, QS, D], u8, tag="xg")
            xgv = dramp.tile([HG, QS, D], bf16, tag="xgv")
            mTg = dramp.tile([S, S // 8], u8, tag="mTg")
            x_scratch = dramp.tile([S, D], bf16, tag="xscr")
            nc.sync.dma_start(wb[:], whalf_v)
            nc.gpsimd.collective_compute(
                "AllGather", mybir.AluOpType.bypass, replica_groups=PAIRS,
                ins=[wb.opt()], outs=[wg.opt()])
            for s in range(2):
                nc.sync.dma_start(xin_b[s], dram_in[f"xsl{s}"][:])
            nc.gpsimd.collective_compute(
                "AllGather", mybir.AluOpType.bypass, replica_groups=GROUPS,
                ins=[xin_b.opt()], outs=[xg.opt()])
            nc.sync.dma_start(xvb[:], dram_in["xsl2"][:])
            nc.gpsimd.collective_compute(
                "AllGather", mybir.AluOpType.bypass, replica_groups=GROUPS,
                ins=[xvb.opt()], outs=[xgv.opt()])
            nc.sync.dma_start(mb[:], maskTp_v)
            nc.gpsimd.collective_compute(
                "AllGather", mybir.AluOpType.bypass, replica_groups=GROUPS,
                ins=[mb.opt()], outs=[mTg.opt()])

            ident = constp.tile([P, P], f32)
            make_identity(nc, ident)
            identb = constp.tile([P, P], bf16)
            nc.vector.tensor_copy(identb[:], ident[:])

            # weights to SBUF  [128, kt, DHG] etc. (from gathered wg)
            w_sb = {}
            for wi, nm in enumerate(["wq", "wk", "wv"]):
                t = persist.tile([P, D // P, DHG], bf16, tag=f"w_{nm}", name=f"w_{nm}")
                nc.sync.dma_start(t[:], wg[wi].rearrange(
                    "(kt p n) -> p kt n", p=P, n=DHG))
                w_sb[nm] = t
            wo_sb = persist.tile([P, DHG // P, D], bf16, tag="w_wo")
            nc.sync.dma_start(wo_sb[:], wg[3].rearrange(
                "(kt p n) -> p kt n", p=P, n=D))
            c_sb = {}
            for ci, nm in enumerate(["cq", "ck", "cv"]):
                t = persist.tile([2, DHG], bf16, tag=f"c_{nm}", name=f"c_{nm}")
                nc.sync.dma_start(
                    t[:], cpk_v[:, ci * DHG:(ci + 1) * DHG])
                c_sb[nm] = t

            # persistent activation tensors
            qT = persist.tile([P, DHG // P, S], bf16, tag="qT")   # Q^T [dk, tok]
            kT = persist.tile([P, DHG // P, S], bf16, tag="kT")   # K^T [dk, tok]
            vhat = persist.tile([P, NTT, HPG, DK + 1], bf16, tag="vhat")
            nc.vector.memset(vhat[:], 0.0)
            nc.vector.memset(vhat[:, :, :, DK:DK + 1], 1.0)
            rows = persist.tile([2, S], bf16, tag="rows")         # [-mean; ones]
            nc.vector.memset(rows[:], 1.0)
            rinv_bc = {}
            for nm in ["k", "q"]:
                rinv_bc[nm] = persist.tile([P, S], f32, tag=f"rinvbc_{nm}", name=f"rinvbc_{nm}")

            # ---------------- Phase A: stats + transpose + projections -------
            for idx, (s_idx, wnm, cnm) in enumerate([
                    (0, "wq", "cq"), (1, "wk", "ck"), (2, "wv", "cv")]):
                with tc.tile_pool(name=f"pa_{idx}", bufs=3) as pa, \
                     tc.tile_pool(name=f"paps_{idx}", bufs=2, space="PSUM") as paps, \
                     tc.tile_pool(name=f"pap2_{idx}", bufs=3, space="PSUM") as pap2:
                    rinv_row = pa.tile([1, S], f32, tag="rinv_row", bufs=1)
                    rinv_cols = pa.tile([P, NTT], f32, tag="rinv_cols", bufs=1)
                    for tt in range(NTT):
                        qs_i, r_i = tt // HG, tt % HG
                        xt = pa.tile([P, D], bf16, tag="xt")
                        if idx < 2:
                            x8 = pa.tile([P, D], u8, tag="x8")
                            nc.sync.dma_start(
                                x8[:], xg[r_i, s_idx, qs_i * P:(qs_i + 1) * P, :])
                            nc.vector.tensor_copy(xt[:], x8[:])
                            nc.sync.dma_start(
                                x_scratch[tt * P:(tt + 1) * P, :], xt[:])
                        else:
                            nc.sync.dma_start(
                                xt[:], xgv[r_i, qs_i * P:(qs_i + 1) * P, :])
                        # LN stats
                        st = pa.tile([P, 2, 6], f32, tag="bnst")
                        xr = xt[:].rearrange("p (n f) -> p n f", f=512)
                        nc.vector.bn_stats(out=st[:, 0], in_=xr[:, 0])
                        nc.vector.bn_stats(out=st[:, 1], in_=xr[:, 1])
                        mv = pa.tile([P, 2], f32, tag="mv")
                        nc.vector.bn_aggr(out=mv[:], in_=st[:])
                        pack = pa.tile([P, 2], f32, tag="pack")
                        # pack[:,0] = -mean ; pack[:,1] = 1/(sqrt(var*c)+eps)
                        nc.vector.tensor_scalar(out=pack[:, 0:1], in0=mv[:, 0:1],
                                                scalar1=-1.0, scalar2=None,
                                                op0=mybir.AluOpType.mult)
                        sd = pa.tile([P, 1], f32, tag="sd")
                        nc.scalar.activation(sd[:], mv[:, 1:2],
                                             mybir.ActivationFunctionType.Sqrt,
                                             scale=VAR_C)
                        nc.vector.tensor_scalar(out=sd[:], in0=sd[:],
                                                scalar1=EPS, scalar2=None,
                                                op0=mybir.AluOpType.add)
                        nc.vector.reciprocal(pack[:, 1:2], sd[:])
                        nc.gpsimd.tensor_copy(rinv_cols[:, tt:tt + 1], pack[:, 1:2])
                        # transpose stats to rows (two base-0 transposes)
                        pst0 = pap2.tile([1, P], f32, tag="pst0")
                        nc.tensor.transpose(pst0[:], pack[:, 0:1], ident[:])
                        nc.scalar.copy(rows[0:1, tt * P:(tt + 1) * P], pst0[:])
                        pst1 = pap2.tile([1, P], f32, tag="pst1")
                        nc.tensor.transpose(pst1[:], pack[:, 1:2], ident[:])
                        nc.scalar.copy(rinv_row[:, tt * P:(tt + 1) * P], pst1[:])
                    # x^T via DMA-transpose (scratch for u8 streams,
                    # direct per-block from gathered bf16 for V)
                    xT = pa.tile([P, D // P, S], bf16, tag="xT", bufs=1)
                    if idx < 2:
                        for kt in range(D // P):
                            nc.sync.dma_start(
                                xT[:, kt], x_scratch[:, kt * P:(kt + 1) * P],
                                transpose=True)
                    else:
                        for tt in range(NTT):
                            qs_i, r_i = tt // HG, tt % HG
                            for kt in range(D // P):
                                nc.sync.dma_start(
                                    xT[:, kt, tt * P:(tt + 1) * P],
                                    xgv[r_i, qs_i * P:(qs_i + 1) * P,
                                        kt * P:(kt + 1) * P],
                                    transpose=True)
                    if idx < 2:
                        # rinv broadcast tile for Q/K evac
                        nc.gpsimd.partition_broadcast(
                            rinv_bc["k" if idx == 0 else "q"][:], rinv_row[:])
                        # projection -> feature-major [dk, tok]
                        dstT = qT if idx == 0 else kT
                        for m in range(DHG // P):
                            for nn in range(NQS):
                                ps = paps.tile([P, QS], f32, tag="projps")
                                for kt in range(D // P):
                                    nc.tensor.matmul(
                                        ps[:],
                                        w_sb[wnm][:, kt, m * P:(m + 1) * P],
                                        xT[:, kt, nn * QS:(nn + 1) * QS],
                                        start=(kt == 0), stop=False)
                                nc.tensor.matmul(
                                    ps[:], c_sb[cnm][:, m * P:(m + 1) * P],
                                    rows[:, nn * QS:(nn + 1) * QS],
                                    start=False, stop=True)
                                nc.vector.tensor_mul(
                                    dstT[:, m, nn * QS:(nn + 1) * QS], ps[:],
                                    rinv_bc["k" if idx == 0 else "q"][:, nn * QS:(nn + 1) * QS])
                    else:
                        # V projection -> token-major [tok, dk], scaled by rinv col
                        for m in range(NTT):
                            ps = paps.tile([P, DHG], f32, tag="projps")
                            for kt in range(D // P):
                                nc.tensor.matmul(
                                    ps[:], xT[:, kt, m * P:(m + 1) * P],
                                    w_sb[wnm][:, kt, :],
                                    start=(kt == 0), stop=False)
                            nc.tensor.matmul(
                                ps[:], rows[:, m * P:(m + 1) * P], c_sb[cnm][:],
                                start=False, stop=True)
                            nc.vector.tensor_scalar(
                                out=vhat[:, m, :, 0:DK],
                                in0=ps[:].rearrange("p (h d) -> p h d", h=HPG),
                                scalar1=rinv_cols[:, m:m + 1], scalar2=None,
                                op0=mybir.AluOpType.mult)

            # ---------------- Phase B: attention + Wo + RS -------------------
            bounce = []
            rs_out = []
            for c in range(NQS):
                bounce.append(dramp.tile([QS, D], f32, tag=f"bounce{c}", name=f"bounce{c}"))
                rs_out.append(dramp.tile([P, D], f32, tag=f"rsout{c}", name=f"rsout{c}"))

            mTg_p = mTg.rearrange("(st p) qb -> p st qb", p=P)

            with tc.tile_pool(name="mk", bufs=2) as mkp, \
                 tc.tile_pool(name="pstr", bufs=2) as pstrp, \
                 tc.tile_pool(name="ctx", bufs=1) as ctxp, \
                 tc.tile_pool(name="att_sc", bufs=2, space="PSUM") as scps, \
                 tc.tile_pool(name="att_pv", bufs=1, space="PSUM") as pvps, \
                 tc.tile_pool(name="att_d", bufs=1, space="PSUM") as dps, \
                 tc.tile_pool(name="att_wo", bufs=1, space="PSUM") as wops, \
                 tc.tile_pool(name="ostage", bufs=3) as ostage, \
                 tc.tile_pool(name="post", bufs=1) as postp:

                ctxT = ctxp.tile([P, DHG // P, S], bf16)

                QB = QS // 8
                for qs in range(NQS):
                    mT8 = mkp.tile([P, NTT, QB], u8, tag="mask8", bufs=1)
                    nc.sync.dma_start(mT8[:], mTg_p[:, :, qs * QB:(qs + 1) * QB])
                    mTu = mkp.tile([P, NTT, QB, 8], u8, tag="masku", bufs=1)
                    for bit in range(8):
                        nc.vector.tensor_scalar(
                            out=mTu[:, :, :, bit], in0=mT8[:],
                            scalar1=bit, scalar2=1,
                            op0=mybir.AluOpType.logical_shift_right,
                            op1=mybir.AluOpType.bitwise_and)
                    mT = mkp.tile([P, NTT, QS], bf16, tag="maskT")
                    nc.vector.tensor_copy(
                        mT[:].rearrange("p t q -> p (t q)"),
                        mTu[:].rearrange("p t b e -> p (t b e)"))
                    for hp in range(2):
                        pstr2 = [pstrp.tile([P, NTT, QS], bf16, tag=f"pstr{i}", name=f"pstr{i}")
                                 for i in range(2)]
                        for st in range(NTT):
                            scs = [scps.tile([P, QS], f32, tag=f"scps{i}", name=f"scps{i}")
                                   for i in range(2)]
                            for hin in range(2):
                                nc.tensor.matmul(
                                    scs[hin][:],
                                    kT[hin * 64:(hin + 1) * 64, hp,
                                       st * P:(st + 1) * P],
                                    qT[hin * 64:(hin + 1) * 64, hp,
                                       qs * QS:(qs + 1) * QS],
                                    start=True, stop=True,
                                    tile_position=(hin * 64, 0))
                            for hin in range(2):
                                nc.scalar.activation(
                                    pstr2[hin][:, st], scs[hin][:],
                                    mybir.ActivationFunctionType.Exp,
                                    scale=1.0 / math.sqrt(DK))
                        for hin in range(2):
                            pstr = pstr2[hin]
                            h = hp * 2 + hin
                            # mask the whole strip in one op
                            nc.vector.tensor_mul(
                                pstr[:].rearrange("p t q -> p (t q)"),
                                pstr[:].rearrange("p t q -> p (t q)"),
                                mT[:].rearrange("p t q -> p (t q)"))
                            # PV with ones column -> [65, QS]
                            pv = pvps.tile([DK + 1, QS], f32, tag="pvps")
                            for st in range(NTT):
                                nc.tensor.matmul(
                                    pv[:],
                                    vhat[:, st, h, :],
                                    pstr[:, st],
                                    start=(st == 0), stop=(st == NTT - 1))
                            ce = ostage.tile([DK + 1, QS], f32, tag="ce")
                            nc.scalar.copy(ce[:], pv[:])
                            # normalize + re-transpose into ctxT
                            for blk in range(QS // P):
                                pt = dps.tile([P, DK + 1], f32, tag="dpt")
                                nc.tensor.transpose(
                                    pt[:], ce[:, blk * P:(blk + 1) * P],
                                    ident[0:DK + 1, 0:DK + 1])
                                rec = ostage.tile([P, 1], f32, tag="rec")
                                nc.vector.reciprocal(rec[:], pt[:, DK:DK + 1])
                                ctok = ostage.tile([P, DK], bf16, tag="ctok")
                                nc.scalar.activation(
                                    ctok[:], pt[:, 0:DK],
                                    mybir.ActivationFunctionType.Copy, scale=rec[:])
                                pb = dps.tile([DK, P], bf16, tag="dpb")
                                nc.tensor.transpose(pb[:], ctok[:], identb[:])
                                nc.scalar.copy(
                                    ctxT[hin * 64:hin * 64 + DK, hp,
                                         qs * QS + blk * P: qs * QS + (blk + 1) * P],
                                    pb[:])
                    # Wo for this q-slice
                    for t4 in range(QS // P):
                        tok0 = qs * QS + t4 * P
                        for nn in range(2):
                            wp = wops.tile([P, 512], f32, tag="wops")
                            for kt in range(DHG // P):
                                nc.tensor.matmul(
                                    wp[:],
                                    ctxT[:, kt, tok0:tok0 + P],
                                    wo_sb[:, kt, nn * 512:(nn + 1) * 512],
                                    start=(kt == 0), stop=(kt == DHG // P - 1))
                            ost = ostage.tile([P, 512], f32, tag="ost")
                            nc.scalar.copy(ost[:], wp[:])
                            nc.sync.dma_start(
                                bounce[qs][t4 * P:(t4 + 1) * P,
                                           nn * 512:(nn + 1) * 512], ost[:])
                    nc.gpsimd.collective_compute(
                        "ReduceScatter", mybir.AluOpType.add,
                        replica_groups=GROUPS,
                        ins=[bounce[qs].opt()], outs=[rs_out[qs].opt()])
                    # post-RS: residual vn for own rows + optional affine
                    vs = postp.tile([P, D], bf16, tag="vs")
                    nc.sync.dma_start(vs[:], dram_in["xsl2"][qs * P:(qs + 1) * P, :])
                    st2 = postp.tile([P, 2, 6], f32, tag="bn2")
                    vr = vs[:].rearrange("p (n f) -> p n f", f=512)
                    nc.vector.bn_stats(out=st2[:, 0], in_=vr[:, 0])
                    nc.vector.bn_stats(out=st2[:, 1], in_=vr[:, 1])
                    mv2 = postp.tile([P, 2], f32, tag="mv2")
                    nc.vector.bn_aggr(out=mv2[:], in_=st2[:])
                    sd2 = postp.tile([P, 1], f32, tag="sd2")
                    nc.scalar.activation(sd2[:], mv2[:, 1:2],
                                         mybir.ActivationFunctionType.Sqrt,
                                         scale=VAR_C)
                    nc.vector.tensor_scalar(out=sd2[:], in0=sd2[:], scalar1=EPS,
                                            scalar2=None, op0=mybir.AluOpType.add)
                    ri2 = postp.tile([P, 1], f32, tag="ri2")
                    nc.vector.reciprocal(ri2[:], sd2[:])
                    nm2 = postp.tile([P, 1], f32, tag="nm2")
                    nc.vector.tensor_scalar(out=nm2[:], in0=mv2[:, 0:1],
                                            scalar1=-1.0, scalar2=None,
                                            op0=mybir.AluOpType.mult)
                    vn = postp.tile([P, D], f32, tag="vn")
                    nc.vector.tensor_scalar(out=vn[:], in0=vs[:],
                                            scalar1=nm2[:], scalar2=ri2[:],
                                            op0=mybir.AluOpType.add,
                                            op1=mybir.AluOpType.mult)
                    if has_a2:
                        a2t = postp.tile([P, D], bf16, tag="a2t")
                        nc.sync.dma_start(
                            a2t[:], cpk_v[0:1, 3 * DHG:3 * DHG + D]
                            .partition_broadcast(P))
                        nc.vector.tensor_mul(vn[:], vn[:], a2t[:])
                    ro = postp.tile([P, D], f32, tag="ro")
                    nc.sync.dma_start(ro[:], rs_out[qs][:])
                    if has_bias_out:
                        bot = postp.tile([P, D], bf16, tag="bot")
                        nc.sync.dma_start(
                            bot[:], cpk_v[1:2, 3 * DHG:3 * DHG + D]
                            .partition_broadcast(P))
                        nc.vector.tensor_add(vn[:], vn[:], bot[:])
                    robf = postp.tile([P, D], bf16, tag="robf")
                    nc.vector.tensor_add(robf[:], ro[:], vn[:])
                    nc.sync.dma_start(out_shard[qs], robf[:])

    nc.compile()
    return nc


def _prep_inputs(k, q, v, mask, Wq, bq, Wk, bk, Wv, bv, Wo, bo, a2, b2):
    """Host-side fold + shard. Returns list of per-core input dicts."""
    a2 = np.asarray(a2, np.float32); b2 = np.asarray(b2, np.float32)
    in_maps = []
    w_eff = {}
    for nm, W, bias in [("q", Wq, bq), ("k", Wk, bk), ("v", Wv, bv)]:
        We = (a2[:, None] * np.asarray(W, np.float32))
        be = b2 @ np.asarray(W, np.float32) + np.asarray(bias, np.float32)
        w_eff[nm] = (We, be)
    wsums = {nm: w_eff[nm][0].sum(0) for nm in ["q", "k", "v"]}
    wbf = {nm: w_eff[nm][0].astype(BF) for nm in ["q", "k", "v"]}
    wobf = np.asarray(Wo, np.float32).astype(BF)
    cpks = []
    wpacks = []
    for r in range(HG):
        hsl = slice(r * DHG, (r + 1) * DHG)
        cpk = np.zeros((2, 3 * DHG + D), BF)
        cpk[0, 3 * DHG:] = a2.astype(BF)
        cpk[1, 3 * DHG:] = np.asarray(bo, np.float32).astype(BF)
        wflat = [None] * 4
        wflat[3] = np.ascontiguousarray(wobf[hsl]).reshape(-1)
        for wi, nm in enumerate(["q", "k", "v"]):
            wflat[wi] = np.ascontiguousarray(wbf[nm][:, hsl]).reshape(-1)
            cpk[0, wi * DHG:(wi + 1) * DHG] = wsums[nm][hsl].astype(BF)
            cpk[1, wi * DHG:(wi + 1) * DHG] = w_eff[nm][1][hsl].astype(BF)
        cpks.append(cpk)
        wpacks.append(np.stack(wflat))          # [4, WSZ] = wq;wk;wv;wo
    mbr_all = (np.asarray(mask) != 0).reshape(B, S // 8, 8, S)
    mpk_all = mbr_all[:, :, 0, :].astype(np.uint8)
    for j in range(1, 8):
        mpk_all |= mbr_all[:, :, j, :] << np.uint8(j)   # [B, S//8, S(k)]
    for g in range(B):
        # strided token-quarters: canonical token qs*512 + r*128 + p
        # -> core r's slice row qs*128 + p
        def quant8(arr):
            # per-token scale: LN is invariant to any per-token affine, so
            # the quantization scale+offset never needs to be shipped
            xf = np.asarray(arr[g], np.float32)
            s = 127.0 / np.maximum(np.abs(xf).max(axis=1, keepdims=True),
                                   np.float32(1e-30))
            return (xf * s + np.float32(128.5)).astype(np.uint8)
        xs = np.stack([quant8(arr).reshape(NQS, HG, P, D)
                       for arr in (k, q)])             # [2, qs, r, 128, D]
        xall = np.ascontiguousarray(xs.transpose(2, 0, 1, 3, 4))  # [r,2,qs,128,D]
        vb = np.asarray(v[g], np.float32).astype(BF).reshape(NQS, HG, P, D)
        vall = np.ascontiguousarray(vb.transpose(1, 0, 2, 3))     # [r,qs,128,D]
        # mask^T bit-packed along q (little bit order); pack before the
        # transpose so the transpose moves 8x fewer bytes
        mTp = np.ascontiguousarray(mpk_all[g].T)   # [S(k), S//8]
        for r in range(HG):
            x3 = xall[r].reshape(2, QS, D)
            aux = np.concatenate([
                np.ascontiguousarray(wpacks[r][g * 2:(g + 1) * 2])
                .view(np.uint8).ravel(),
                cpks[r].view(np.uint8).ravel(),
                mTp[r * QS:(r + 1) * QS].ravel(),
            ])
            d = {
                "xsl0": x3[0], "xsl1": x3[1],
                "xsl2": vall[r].reshape(QS, D),
                "aux": aux,
            }
            in_maps.append(d)
    return in_maps


def kernel(k, q, v, mask, Wq, bq, Wk, bk, Wv, bv, Wo, bo, a2, b2):
    k = np.asarray(k, np.float32); q = np.asarray(q, np.float32)
    v = np.asarray(v, np.float32); mask = np.asarray(mask)
    has_a2 = not np.allclose(np.asarray(a2, np.float32), 1.0)
    bias_out = np.asarray(bo, np.float32)
    has_bias_out = bool(np.any(bias_out != 0))
    key = (has_a2, has_bias_out)
    if key not in _CACHE:
        _CACHE[key] = _build(has_a2, has_bias_out)
    nc = _CACHE[key]
    in_maps = _prep_inputs(k, q, v, mask, Wq, bq, Wk, bk, Wv, bv, Wo, bo, a2, b2)
    res = run_bass_kernel_spmd(nc, in_maps, core_ids=list(range(NCORES)))
    out = np.empty((B, S, D), np.float32)
    ci = 0
    for g in range(B):
        for r in range(HG):
            sh = res.results[ci]["out_shard"]   # [NQS, P, D]
            for c in range(NQS):
                out[g, c * QS + r * P: c * QS + (r + 1) * P] = sh[c]
            ci += 1
    return out


if __name__ == "__main__":
    rng = np.random.default_rng(0)
    pass


# revision 35
# speedup vs baseline: 1.1630x; 1.1630x over previous
"""Multi-headed attention (pre-LN, quirk-wired) Trainium2 Bass kernel.

Sharding: 8 cores = 2 batches x 4 head-groups (4 heads each).
I/O-lean contract (host<->device transfer dominates the wall clock, so
ship the information-theoretic minimum): each core ships its strided
token-quarter of k/q as uint8 (LayerNorm is invariant to the per-token
quantization affine, so no scales ship and nothing dequantizes), v as
bf16 (its error hits the output directly via the residual), a
bit-packed mask^T quarter, half of its head-slice weights, and folded
LN/bias constants — small tensors merged into one aux byte-tensor via
bitcast views.  Full per-batch tensors are rebuilt on device with
AllGathers (batch group for x/mask, cross-batch pair for weights).
Canonical 128-token tiles map to contiguous blocks of the gathered
buffer (block b = (tt%4)*4 + tt//4), so the attention math is
unchanged from the dense version.  Output returns as bf16.

Per core: full-batch LN-folded projections (its head slice), scores^T
layout attention with ones-column softmax denominators, Wo partial,
ReduceScatter over the 4-core batch group, residual added post-RS.

reference semantics:
  kn,qn,vn = LN(k),LN(q),LN(v)   (ddof=1 std, eps added to std, affine a2,b2)
  query = kn@Wq+bq ; key = qn@Wk+bk ; value = vn@Wv+bv   (note stream quirk)
  out = softmax(mask(QK^T/8)) @ V  -> @Wo + bo + vn
"""
import math
import numpy as np
import ml_dtypes

import jax as _jax
try:
    _jax.config.update("jax_compilation_cache_dir", "/tmp/jax_cc_cache")
    _jax.config.update("jax_persistent_cache_min_compile_time_secs", 0)
    _jax.config.update("jax_persistent_cache_min_entry_size_bytes", 0)
except Exception:
    pass

import concourse.bass as bass
import concourse.tile as tile
from concourse import bacc, mybir
from concourse.bass_utils import run_bass_kernel_spmd
from concourse.masks import make_identity

BF = ml_dtypes.bfloat16
B, S, D, H = 2, 2048, 1024, 16
DK = D // H            # 64
NCORES = 8
HG = 4                 # head-groups per batch
HPG = H // HG          # 4 heads per core
DHG = HPG * DK         # 256 head-dim slice per core
EPS = 1e-6
P = 128
NTT = S // P           # 16 token tiles
NQS = 4                # query slices of 512
QS = S // NQS          # 512
VAR_C = D / (D - 1.0)  # ddof=1 correction
GROUPS = [[0, 1, 2, 3], [4, 5, 6, 7]]
PAIRS = [[0, 4], [1, 5], [2, 6], [3, 7]]   # same head-slice, both batches
WSZ = D * DHG          # elements of one projection weight slice

_CACHE = {}


def _build(has_a2, has_bias_out):
    nc = bacc.Bacc("TRN2", target_bir_lowering=False, debug=False,
                   num_devices=NCORES)
    f32, bf16 = mybir.dt.float32, mybir.dt.bfloat16
    u8 = mybir.dt.uint8
    dram_in = {}
    for nm, shape, dt in [
        ("xsl0", [QS, D], u8),             # strided token-quarter, k stream
        ("xsl1", [QS, D], u8),             # .. q stream  (uint8: LN absorbs
                                           #  the per-token quant affine)
        ("xsl2", [QS, D], bf16),           # .. v stream (bf16: residual +
                                           #  PV values dominate the error)
        # aux = bf16 [whalf(2,WSZ) | cpk(2,3*DHG+D)] bytes ++ u8 maskTp bytes
        ("aux", [2 * (2 * WSZ + 2 * (3 * DHG + D)) + QS * (S // 8)], u8),
    ]:
        dram_in[nm] = nc.dram_tensor(nm, shape, dt, kind="ExternalInput").ap()
    out_shard = nc.dram_tensor("out_shard", [NQS, P, D], bf16,
                               kind="ExternalOutput").ap()
    NBF = 2 * WSZ + 2 * (3 * DHG + D)      # bf16 elements in aux head
    aux_bf = dram_in["aux"][0:2 * NBF].bitcast(bf16)
    whalf_v = aux_bf[0:2 * WSZ].rearrange("(a b) -> a b", a=2)
    cpk_v = aux_bf[2 * WSZ:NBF].rearrange("(a b) -> a b", a=2)
    maskTp_v = dram_in["aux"][2 * NBF:].rearrange("(a b) -> a b", a=QS)

    with tile.TileContext(nc, trace_sim=False) as tc:
        with tc.tile_pool(name="const", bufs=1) as constp, \
             tc.tile_pool(name="persist", bufs=1) as persist, \
             tc.tile_pool(name="dram", bufs=1, space="DRAM") as dramp:

            # ---- gather weights, full-batch activations, mask on device ---
            wb = dramp.tile([2, WSZ], bf16, tag="wb")
            xin_b = dramp.tile([2, QS, D], u8, tag="xin_b")
            xvb = dramp.tile([QS, D], bf16, tag="xvb")
            mb = dramp.tile([QS, S // 8], u8, tag="mb")
            wg = dramp.tile([4, WSZ], bf16, tag="wg")
            xg = dramp.tile([HG, 2, QS, D], u8, tag="xg")
            xgv = dramp.tile([HG, QS, D], bf16, tag="xgv")
            mTg = dramp.tile([S, S // 8], u8, tag="mTg")
            x_scratch = dramp.tile([S, D], bf16, tag="xscr")
            nc.sync.dma_start(wb[:], whalf_v)
            nc.gpsimd.collective_compute(
                "AllGather", mybir.AluOpType.bypass, replica_groups=PAIRS,
                ins=[wb.opt()], outs=[wg.opt()])
            for s in range(2):
                nc.sync.dma_start(xin_b[s], dram_in[f"xsl{s}"][:])
            nc.gpsimd.collective_compute(
                "AllGather", mybir.AluOpType.bypass, replica_groups=GROUPS,
                ins=[xin_b.opt()], outs=[xg.opt()])
            nc.sync.dma_start(xvb[:], dram_in["xsl2"][:])
            nc.gpsimd.collective_compute(
                "AllGather", mybir.AluOpType.bypass, replica_groups=GROUPS,
                ins=[xvb.opt()], outs=[xgv.opt()])
            nc.sync.dma_start(mb[:], maskTp_v)
            nc.gpsimd.collective_compute(
                "AllGather", mybir.AluOpType.bypass, replica_groups=GROUPS,
                ins=[mb.opt()], outs=[mTg.opt()])

            ident = constp.tile([P, P], f32)
            make_identity(nc, ident)
            identb = constp.tile([P, P], bf16)
            nc.vector.tensor_copy(identb[:], ident[:])

            # weights to SBUF  [128, kt, DHG] etc. (from gathered wg)
            w_sb = {}
            for wi, nm in enumerate(["wq", "wk", "wv"]):
                t = persist.tile([P, D // P, DHG], bf16, tag=f"w_{nm}", name=f"w_{nm}")
                nc.sync.dma_start(t[:], wg[wi].rearrange(
                    "(kt p n) -> p kt n", p=P, n=DHG))
                w_sb[nm] = t
            wo_sb = persist.tile([P, DHG // P, D], bf16, tag="w_wo")
            nc.sync.dma_start(wo_sb[:], wg[3].rearrange(
                "(kt p n) -> p kt n", p=P, n=D))
            c_sb = {}
            for ci, nm in enumerate(["cq", "ck", "cv"]):
                t = persist.tile([2, DHG], bf16, tag=f"c_{nm}", name=f"c_{nm}")
                nc.sync.dma_start(
                    t[:], cpk_v[:, ci * DHG:(ci + 1) * DHG])
                c_sb[nm] = t

            # persistent activation tensors
            qT = persist.tile([P, DHG // P, S], bf16, tag="qT")   # Q^T [dk, tok]
            kT = persist.tile([P, DHG // P, S], bf16, tag="kT")   # K^T [dk, tok]
            vhat = persist.tile([P, NTT, HPG, DK + 1], bf16, tag="vhat")
            nc.vector.memset(vhat[:], 0.0)
            nc.vector.memset(vhat[:, :, :, DK:DK + 1], 1.0)
            rows = persist.tile([2, S], bf16, tag="rows")         # [-mean; ones]
            nc.vector.memset(rows[:], 1.0)
            rinv_bc = {}
            for nm in ["k", "q"]:
                rinv_bc[nm] = persist.tile([P, S], f32, tag=f"rinvbc_{nm}", name=f"rinvbc_{nm}")

            # ---------------- Phase A: stats + transpose + projections -------
            for idx, (s_idx, wnm, cnm) in enumerate([
                    (0, "wq", "cq"), (1, "wk", "ck"), (2, "wv", "cv")]):
                with tc.tile_pool(name=f"pa_{idx}", bufs=3) as pa, \
                     tc.tile_pool(name=f"paps_{idx}", bufs=2, space="PSUM") as paps, \
                     tc.tile_pool(name=f"pap2_{idx}", bufs=3, space="PSUM") as pap2:
                    rinv_row = pa.tile([1, S], f32, tag="rinv_row", bufs=1)
                    rinv_cols = pa.tile([P, NTT], f32, tag="rinv_cols", bufs=1)
                    for tt in range(NTT):
                        qs_i, r_i = tt // HG, tt % HG
                        xt = pa.tile([P, D], bf16, tag="xt")
                        if idx < 2:
                            x8 = pa.tile([P, D], u8, tag="x8")
                            nc.sync.dma_start(
                                x8[:], xg[r_i, s_idx, qs_i * P:(qs_i + 1) * P, :])
                            nc.vector.tensor_copy(xt[:], x8[:])
                            nc.sync.dma_start(
                                x_scratch[tt * P:(tt + 1) * P, :], xt[:])
                        else:
                            nc.sync.dma_start(
                                xt[:], xgv[r_i, qs_i * P:(qs_i + 1) * P, :])
                        # LN stats
                        st = pa.tile([P, 2, 6], f32, tag="bnst")
                        xr = xt[:].rearrange("p (n f) -> p n f", f=512)
                        nc.vector.bn_stats(out=st[:, 0], in_=xr[:, 0])
                        nc.vector.bn_stats(out=st[:, 1], in_=xr[:, 1])
                        mv = pa.tile([P, 2], f32, tag="mv")
                        nc.vector.bn_aggr(out=mv[:], in_=st[:])
                        pack = pa.tile([P, 2], f32, tag="pack")
                        # pack[:,0] = -mean ; pack[:,1] = 1/(sqrt(var*c)+eps)
                        nc.vector.tensor_scalar(out=pack[:, 0:1], in0=mv[:, 0:1],
                                                scalar1=-1.0, scalar2=None,
                                                op0=mybir.AluOpType.mult)
                        sd = pa.tile([P, 1], f32, tag="sd")
                        nc.scalar.activation(sd[:], mv[:, 1:2],
                                             mybir.ActivationFunctionType.Sqrt,
                                             scale=VAR_C)
                        nc.vector.tensor_scalar(out=sd[:], in0=sd[:],
                                                scalar1=EPS, scalar2=None,
                                                op0=mybir.AluOpType.add)
                        nc.vector.reciprocal(pack[:, 1:2], sd[:])
                        nc.gpsimd.tensor_copy(rinv_cols[:, tt:tt + 1], pack[:, 1:2])
                        # transpose stats to rows (two base-0 transposes)
                        pst0 = pap2.tile([1, P], f32, tag="pst0")
                        nc.tensor.transpose(pst0[:], pack[:, 0:1], ident[:])
                        nc.scalar.copy(rows[0:1, tt * P:(tt + 1) * P], pst0[:])
                        pst1 = pap2.tile([1, P], f32, tag="pst1")
                        nc.tensor.transpose(pst1[:], pack[:, 1:2], ident[:])
                        nc.scalar.copy(rinv_row[:, tt * P:(tt + 1) * P], pst1[:])
                    # x^T via DMA-transpose (scratch for u8 streams,
                    # direct per-block from gathered bf16 for V)
                    xT = pa.tile([P, D // P, S], bf16, tag="xT", bufs=1)
                    if idx < 2:
                        for kt in range(D // P):
                            nc.sync.dma_start(
                                xT[:, kt], x_scratch[:, kt * P:(kt + 1) * P],
                                transpose=True)
                    else:
                        for tt in range(NTT):
                            qs_i, r_i = tt // HG, tt % HG
                            for kt in range(D // P):
                                nc.sync.dma_start(
                                    xT[:, kt, tt * P:(tt + 1) * P],
                                    xgv[r_i, qs_i * P:(qs_i + 1) * P,
                                        kt * P:(kt + 1) * P],
                                    transpose=True)
                    if idx < 2:
                        # rinv broadcast tile for Q/K evac
                        nc.gpsimd.partition_broadcast(
                            rinv_bc["k" if idx == 0 else "q"][:], rinv_row[:])
                        # projection -> feature-major [dk, tok]
                        dstT = qT if idx == 0 else kT
                        for m in range(DHG // P):
                            for nn in range(NQS):
                                ps = paps.tile([P, QS], f32, tag="projps")
                                for kt in range(D // P):
                                    nc.tensor.matmul(
                                        ps[:],
                                        w_sb[wnm][:, kt, m * P:(m + 1) * P],
                                        xT[:, kt, nn * QS:(nn + 1) * QS],
                                        start=(kt == 0), stop=False)
                                nc.tensor.matmul(
                                    ps[:], c_sb[cnm][:, m * P:(m + 1) * P],
                                    rows[:, nn * QS:(nn + 1) * QS],
                                    start=False, stop=True)
                                nc.vector.tensor_mul(
                                    dstT[:, m, nn * QS:(nn + 1) * QS], ps[:],
                                    rinv_bc["k" if idx == 0 else "q"][:, nn * QS:(nn + 1) * QS])
                    else:
                        # V projection -> token-major [tok, dk], scaled by rinv col
                        for m in range(NTT):
                            ps = paps.tile([P, DHG], f32, tag="projps")
                            for kt in range(D // P):
                                nc.tensor.matmul(
                                    ps[:], xT[:, kt, m * P:(m + 1) * P],
                                    w_sb[wnm][:, kt, :],
                                    start=(kt == 0), stop=False)
                            nc.tensor.matmul(
                                ps[:], rows[:, m * P:(m + 1) * P], c_sb[cnm][:],
                                start=False, stop=True)
                            nc.vector.tensor_scalar(
                                out=vhat[:, m, :, 0:DK],
                                in0=ps[:].rearrange("p (h d) -> p h d", h=HPG),
                                scalar1=rinv_cols[:, m:m + 1], scalar2=None,
                                op0=mybir.AluOpType.mult)

            # ---------------- Phase B: attention + Wo + RS -------------------
            bounce = []
            rs_out = []
            for c in range(NQS):
                bounce.append(dramp.tile([QS, D], f32, tag=f"bounce{c}", name=f"bounce{c}"))
                rs_out.append(dramp.tile([P, D], f32, tag=f"rsout{c}", name=f"rsout{c}"))

            mTg_p = mTg.rearrange("(st p) qb -> p st qb", p=P)

            with tc.tile_pool(name="mk", bufs=2) as mkp, \
                 tc.tile_pool(name="pstr", bufs=2) as pstrp, \
                 tc.tile_pool(name="ctx", bufs=1) as ctxp, \
                 tc.tile_pool(name="att_sc", bufs=2, space="PSUM") as scps, \
                 tc.tile_pool(name="att_pv", bufs=1, space="PSUM") as pvps, \
                 tc.tile_pool(name="att_d", bufs=1, space="PSUM") as dps, \
                 tc.tile_pool(name="att_wo", bufs=1, space="PSUM") as wops, \
                 tc.tile_pool(name="ostage", bufs=3) as ostage, \
                 tc.tile_pool(name="post", bufs=1) as postp:

                ctxT = ctxp.tile([P, DHG // P, S], bf16)

                QB = QS // 8
                for qs in range(NQS):
                    mT8 = mkp.tile([P, NTT, QB], u8, tag="mask8", bufs=1)
                    nc.sync.dma_start(mT8[:], mTg_p[:, :, qs * QB:(qs + 1) * QB])
                    mTu = mkp.tile([P, NTT, QB, 8], u8, tag="masku", bufs=1)
                    for bit in range(8):
                        nc.vector.tensor_scalar(
                            out=mTu[:, :, :, bit], in0=mT8[:],
                            scalar1=bit, scalar2=1,
                            op0=mybir.AluOpType.logical_shift_right,
                            op1=mybir.AluOpType.bitwise_and)
                    mT = mkp.tile([P, NTT, QS], bf16, tag="maskT")
                    nc.vector.tensor_copy(
                        mT[:].rearrange("p t q -> p (t q)"),
                        mTu[:].rearrange("p t b e -> p (t b e)"))
                    for hp in range(2):
                        pstr2 = [pstrp.tile([P, NTT, QS], bf16, tag=f"pstr{i}", name=f"pstr{i}")
                                 for i in range(2)]
                        for st in range(NTT):
                            scs = [scps.tile([P, QS], f32, tag=f"scps{i}", name=f"scps{i}")
                                   for i in range(2)]
                            for hin in range(2):
                                nc.tensor.matmul(
                                    scs[hin][:],
                                    kT[hin * 64:(hin + 1) * 64, hp,
                                       st * P:(st + 1) * P],
                                    qT[hin * 64:(hin + 1) * 64, hp,
                                       qs * QS:(qs + 1) * QS],
                                    start=True, stop=True,
                                    tile_position=(hin * 64, 0))
                            for hin in range(2):
                                nc.scalar.activation(
                                    pstr2[hin][:, st], scs[hin][:],
                                    mybir.ActivationFunctionType.Exp,
                                    scale=1.0 / math.sqrt(DK))
                        for hin in range(2):
                            pstr = pstr2[hin]
                            h = hp * 2 + hin
                            # mask the whole strip in one op
                            nc.vector.tensor_mul(
                                pstr[:].rearrange("p t q -> p (t q)"),
                                pstr[:].rearrange("p t q -> p (t q)"),
                                mT[:].rearrange("p t q -> p (t q)"))
                            # PV with ones column -> [65, QS]
                            pv = pvps.tile([DK + 1, QS], f32, tag="pvps")
                            for st in range(NTT):
                                nc.tensor.matmul(
                                    pv[:],
                                    vhat[:, st, h, :],
                                    pstr[:, st],
                                    start=(st == 0), stop=(st == NTT - 1))
                            ce = ostage.tile([DK + 1, QS], f32, tag="ce")
                            nc.scalar.copy(ce[:], pv[:])
                            # normalize + re-transpose into ctxT
                            for blk in range(QS // P):
                                pt = dps.tile([P, DK + 1], f32, tag="dpt")
                                nc.tensor.transpose(
                                    pt[:], ce[:, blk * P:(blk + 1) * P],
                                    ident[0:DK + 1, 0:DK + 1])
                                rec = ostage.tile([P, 1], f32, tag="rec")
                                nc.vector.reciprocal(rec[:], pt[:, DK:DK + 1])
                                ctok = ostage.tile([P, DK], bf16, tag="ctok")
                                nc.scalar.activation(
                                    ctok[:], pt[:, 0:DK],
                                    mybir.ActivationFunctionType.Copy, scale=rec[:])
                                pb = dps.tile([DK, P], bf16, tag="dpb")
                                nc.tensor.transpose(pb[:], ctok[:], identb[:])
                                nc.scalar.copy(
                                    ctxT[hin * 64:hin * 64 + DK, hp,
                                         qs * QS + blk * P: qs * QS + (blk + 1) * P],
                                    pb[:])
                    # Wo for this q-slice
                    for t4 in range(QS // P):
                        tok0 = qs * QS + t4 * P
                        for nn in range(2):
                            wp = wops.tile([P, 512], f32, tag="wops")
                            for kt in range(DHG // P):
                                nc.tensor.matmul(
                                    wp[:],
                                    ctxT[:, kt, tok0:tok0 + P],
                                    wo_sb[:, kt, nn * 512:(nn + 1) * 512],
                                    start=(kt == 0), stop=(kt == DHG // P - 1))
                            ost = ostage.tile([P, 512], f32, tag="ost")
                            nc.scalar.copy(ost[:], wp[:])
                            nc.sync.dma_start(
                                bounce[qs][t4 * P:(t4 + 1) * P,
                                           nn * 512:(nn + 1) * 512], ost[:])
                    nc.gpsimd.collective_compute(
                        "ReduceScatter", mybir.AluOpType.add,
                        replica_groups=GROUPS,
                        ins=[bounce[qs].opt()], outs=[rs_out[qs].opt()])
                    # post-RS: residual vn for own rows + optional affine
                    vs = postp.tile([P, D], bf16, tag="vs")
                    nc.sync.dma_start(vs[:], dram_in["xsl2"][qs * P:(qs + 1) * P, :])
                    st2 = postp.tile([P, 2, 6], f32, tag="bn2")
                    vr = vs[:].rearrange("p (n f) -> p n f", f=512)
                    nc.vector.bn_stats(out=st2[:, 0], in_=vr[:, 0])
                    nc.vector.bn_stats(out=st2[:, 1], in_=vr[:, 1])
                    mv2 = postp.tile([P, 2], f32, tag="mv2")
                    nc.vector.bn_aggr(out=mv2[:], in_=st2[:])
                    sd2 = postp.tile([P, 1], f32, tag="sd2")
                    nc.scalar.activation(sd2[:], mv2[:, 1:2],
                                         mybir.ActivationFunctionType.Sqrt,
                                         scale=VAR_C)
                    nc.vector.tensor_scalar(out=sd2[:], in0=sd2[:], scalar1=EPS,
                                            scalar2=None, op0=mybir.AluOpType.add)
                    ri2 = postp.tile([P, 1], f32, tag="ri2")
                    nc.vector.reciprocal(ri2[:], sd2[:])
                    nm2 = postp.tile([P, 1], f32, tag="nm2")
                    nc.vector.tensor_scalar(out=nm2[:], in0=mv2[:, 0:1],
                                            scalar1=-1.0, scalar2=None,
                                            op0=mybir.AluOpType.mult)
                    vn = postp.tile([P, D], f32, tag="vn")
                    nc.vector.tensor_scalar(out=vn[:], in0=vs[:],
                                            scalar1=nm2[:], scalar2=ri2[:],
                                            op0=mybir.AluOpType.add,
                                            op1=mybir.AluOpType.mult)
                    if has_a2:
                        a2t = postp.tile([P, D], bf16, tag="a2t")
                        nc.sync.dma_start(
                            a2t[:], cpk_v[0:1, 3 * DHG:3 * DHG + D]
                            .partition_broadcast(P))
                        nc.vector.tensor_mul(vn[:], vn[:], a2t[:])
                    ro = postp.tile([P, D], f32, tag="ro")
                    nc.sync.dma_start(ro[:], rs_out[qs][:])
                    if has_bias_out:
                        bot = postp.tile([P, D], bf16, tag="bot")
                        nc.sync.dma_start(
                            bot[:], cpk_v[1:2, 3 * DHG:3 * DHG + D]
                            .partition_broadcast(P))
                        nc.vector.tensor_add(vn[:], vn[:], bot[:])
                    robf = postp.tile([P, D], bf16, tag="robf")
                    nc.vector.tensor_add(robf[:], ro[:], vn[:])
                    nc.sync.dma_start(out_shard[qs], robf[:])

    nc.compile()
    return nc


def _prep_inputs(k, q, v, mask, Wq, bq, Wk, bk, Wv, bv, Wo, bo, a2, b2):
    """Host-side fold + shard. Returns list of per-core input dicts."""
    a2 = np.asarray(a2, np.float32); b2 = np.asarray(b2, np.float32)
    in_maps = []
    w_eff = {}
    for nm, W, bias in [("q", Wq, bq), ("k", Wk, bk), ("v", Wv, bv)]:
        Wf = np.asarray(W, np.float32)
        We = Wf if np.all(a2 == 1.0) else a2[:, None] * Wf
        be = (np.asarray(bias, np.float32) if not np.any(b2)
              else b2 @ Wf + np.asarray(bias, np.float32))
        w_eff[nm] = (We, be)
    wsums = {nm: w_eff[nm][0].sum(0) for nm in ["q", "k", "v"]}
    wbf = {nm: w_eff[nm][0].astype(BF) for nm in ["q", "k", "v"]}
    wobf = np.asarray(Wo, np.float32).astype(BF)
    cpks = []
    wpacks = []
    for r in range(HG):
        hsl = slice(r * DHG, (r + 1) * DHG)
        cpk = np.zeros((2, 3 * DHG + D), BF)
        cpk[0, 3 * DHG:] = a2.astype(BF)
        cpk[1, 3 * DHG:] = np.asarray(bo, np.float32).astype(BF)
        wflat = [None] * 4
        wflat[3] = np.ascontiguousarray(wobf[hsl]).reshape(-1)
        for wi, nm in enumerate(["q", "k", "v"]):
            wflat[wi] = np.ascontiguousarray(wbf[nm][:, hsl]).reshape(-1)
            cpk[0, wi * DHG:(wi + 1) * DHG] = wsums[nm][hsl].astype(BF)
            cpk[1, wi * DHG:(wi + 1) * DHG] = w_eff[nm][1][hsl].astype(BF)
        cpks.append(cpk)
        wpacks.append(np.stack(wflat))          # [4, WSZ] = wq;wk;wv;wo
    mbr_all = (np.asarray(mask) != 0).reshape(B, S // 8, 8, S)
    mpk_all = mbr_all[:, :, 0, :].astype(np.uint8)
    for j in range(1, 8):
        mpk_all |= mbr_all[:, :, j, :] << np.uint8(j)   # [B, S//8, S(k)]
    for g in range(B):
        # strided token-quarters: canonical token qs*512 + r*128 + p
        # -> core r's slice row qs*128 + p
        def quant8(arr):
            # per-token scale: LN is invariant to any per-token affine, so
            # the quantization scale+offset never needs to be shipped
            xf = np.asarray(arr[g], np.float32)
            s = 127.0 / np.maximum(np.abs(xf).max(axis=1, keepdims=True),
                                   np.float32(1e-30))
            return (xf * s + np.float32(128.5)).astype(np.uint8)
        xs = np.stack([quant8(arr).reshape(NQS, HG, P, D)
                       for arr in (k, q)])             # [2, qs, r, 128, D]
        xall = np.ascontiguousarray(xs.transpose(2, 0, 1, 3, 4))  # [r,2,qs,128,D]
        vb = np.asarray(v[g], np.float32).astype(BF).reshape(NQS, HG, P, D)
        vall = np.ascontiguousarray(vb.transpose(1, 0, 2, 3))     # [r,qs,128,D]
        # mask^T bit-packed along q (little bit order); pack before the
        # transpose so the transpose moves 8x fewer bytes
        mTp = np.ascontiguousarray(mpk_all[g].T)   # [S(k), S//8]
        for r in range(HG):
            x3 = xall[r].reshape(2, QS, D)
            aux = np.concatenate([
                np.ascontiguousarray(wpacks[r][g * 2:(g + 1) * 2])
                .view(np.uint8).ravel(),
                cpks[r].view(np.uint8).ravel(),
                mTp[r * QS:(r + 1) * QS].ravel(),
            ])
            d = {
                "xsl0": x3[0], "xsl1": x3[1],
                "xsl2": vall[r].reshape(QS, D),
                "aux": aux,
            }
            in_maps.append(d)
    return in_maps


def kernel(k, q, v, mask, Wq, bq, Wk, bk, Wv, bv, Wo, bo, a2, b2):
    k = np.asarray(k, np.float32); q = np.asarray(q, np.float32)
    v = np.asarray(v, np.float32); mask = np.asarray(mask)
    has_a2 = not np.allclose(np.asarray(a2, np.float32), 1.0)
    bias_out = np.asarray(bo, np.float32)
    has_bias_out = bool(np.any(bias_out != 0))
    key = (has_a2, has_bias_out)
    if key not in _CACHE:
        _CACHE[key] = _build(has_a2, has_bias_out)
    nc = _CACHE[key]
    in_maps = _prep_inputs(k, q, v, mask, Wq, bq, Wk, bk, Wv, bv, Wo, bo, a2, b2)
    res = run_bass_kernel_spmd(nc, in_maps, core_ids=list(range(NCORES)))
    out = np.empty((B, S, D), np.float32)
    ci = 0
    for g in range(B):
        for r in range(HG):
            sh = res.results[ci]["out_shard"]   # [NQS, P, D]
            for c in range(NQS):
                out[g, c * QS + r * P: c * QS + (r + 1) * P] = sh[c]
            ci += 1
    return out


if __name__ == "__main__":
    rng = np.random.default_rng(0)
    pass
